# revision 8
# baseline (speedup 1.0000x reference)
"""Trainium2 Bass kernel for nn_Decoder_90443421319294.

Sharding: 2 batches x 4-way sequence-parallel over query rows (8 NeuronCores).
Within each 4-core group, K and V^T are computed on row shards and AllGathered;
attention (S = Q^T K, unscaled softmax over keys, O = A @ V^T) runs flash-style
over 4 key quarters with online softmax. InstanceNorm stats use AllReduce.
All matmuls use fp16 inputs with fp32 PSUM accumulation (validated: ~7.5e-3
relative absmax vs fp64 reference end-to-end).

Self-contained: hardcodes all shapes; call kernel(**setup_inputs()).
"""
import numpy as np

N = 6890          # real positions
NP = 7168         # padded positions (56 * 128)
NL = 1792         # positions per core (NP / 4)
QT = 14           # query tiles of 128 per core
B = 2
CS = {1: 1024, 2: 512, 3: 512, 4: 256}
NCH = [(0, 512), (512, 512), (1024, 512), (1536, 256)]  # 512-chunks of 1792
PADL = N - 3 * NL   # 1514: first pad column (local) in the 4th quarter
GROUPS = [[0, 1, 2, 3], [4, 5, 6, 7]]

_cache = {}


def _build():
    if "nc" in _cache:
        return _cache["nc"]
    import concourse.bacc as bacc
    import concourse.mybir as mybir
    import concourse.tile as tile

    F32 = mybir.dt.float32
    F16 = mybir.dt.float16
    AF = mybir.ActivationFunctionType
    ALU = mybir.AluOpType
    AX = mybir.AxisListType

    nc = bacc.Bacc("TRN2", target_bir_lowering=False, debug=False, num_devices=8)

    din = {}

    def inp(name, shape, dt):
        din[name] = nc.dram_tensor(name, shape, dt, kind="ExternalInput")

    inp("x1s", [1024, NL], F16)
    inp("x2s", [1024, NL], F16)
    inp("adds", [3, NL], F16)
    inp("maskh", [128, NL], F16)
    inp("ident", [128, 128], F16)
    for i, (ci, co) in enumerate([(1024, 1024), (1024, 512), (512, 512), (512, 256), (256, 3)], 1):
        inp(f"w_conv{i}", [ci, co], F16)
        inp(f"b_conv{i}", [128, max(1, co // 128)], F32)
    for s in (1, 2, 3, 4):
        C = CS[s]
        cc = C // 128
        for t in ("q", "k", "v"):
            inp(f"w_{t}{s}", [C, C], F16)
        inp(f"b_q{s}", [128, cc], F32)
        inp(f"b_k{s}", [128, cc], F32)
        inp(f"gbv{s}", [128, cc], F32)
        inp(f"gamma{s}", [128, 1], F32)
        for t in ("sp1w", "sp1b", "sp2w", "sp2b", "sprw", "sprb"):
            inp(f"w_{t}{s}", [3, C], F16)
            inp(f"b_{t}{s}", [128, cc], F32)
        for t in ("rc1", "rc2", "rcr"):
            inp(f"w_{t}{s}", [C, C], F16)
            inp(f"b_{t}{s}", [128, cc], F32)
    out_d = nc.dram_tensor("out", [3, NL], F32, kind="ExternalOutput")

    with tile.TileContext(nc) as tc:
        with (
            tc.tile_pool(name="cn", bufs=1) as cn,
            tc.tile_pool(name="wp", bufs=2) as wp,
            tc.tile_pool(name="rg", bufs=1) as rg,
            tc.tile_pool(name="s1", bufs=1) as s1p,
            tc.tile_pool(name="s2", bufs=2) as s2p,
            tc.tile_pool(name="st", bufs=1) as st,
            tc.tile_pool(name="pp", bufs=1, space="PSUM") as pp,
            tc.tile_pool(name="pc", bufs=2, space="PSUM") as pc,
            tc.tile_pool(name="dr", bufs=1, space="DRAM") as dr,
        ):
            ident = cn.tile([128, 128], F16, tag="ident")
            nc.sync.dma_start(ident[:], din["ident"][:])
            maskh = cn.tile([128, NL], F16, tag="maskh")
            nc.sync.dma_start(maskh[:], din["maskh"][:])
            adds = cn.tile([3, NL], F16, tag="adds")
            nc.sync.dma_start(adds[:], din["adds"][:])

            def load_b(name, cc):
                b = st.tile([128, cc], F32, tag=f"B_{name}")
                nc.sync.dma_start(b[:], din[name][:])
                return b

            def conv(w_name, b_name, x, ci_t, co_t, out, relu=False, out_dram=None):
                """x [128, ci_t, NL] f16 -> out [128, co_t, NL] f16 (SBUF tile),
                or stream per-co to out_dram [128, co_t, NL] via small buffers."""
                b = load_b(b_name, co_t) if b_name else None
                for co in range(co_t):
                    w = wp.tile([128, ci_t, 128], F16, tag="W")
                    nc.sync.dma_start(
                        w[:], din[w_name][:, co * 128:(co + 1) * 128]
                        .rearrange("(t p) o -> p t o", p=128))
                    if out_dram is not None:
                        ot = s2p.tile([128, NL], F16, tag="KEV")
                    for off, wd in NCH:
                        ps = pc.tile([128, 512], F32, tag="CV")
                        for ci in range(ci_t):
                            nc.tensor.matmul(
                                ps[:, :wd], w[:, ci, :], x[:, ci, off:off + wd],
                                start=(ci == 0), stop=(ci == ci_t - 1))
                        dst = ot[:, off:off + wd] if out_dram is not None else out[:, co, off:off + wd]
                        if b is None:
                            nc.scalar.activation(dst, ps[:, :wd], AF.Relu if relu else AF.Copy)
                        else:
                            nc.scalar.activation(dst, ps[:, :wd],
                                                 AF.Relu if relu else AF.Identity,
                                                 bias=b[:, co:co + 1])
                    if out_dram is not None:
                        nc.sync.dma_start(out_dram[:, co, :], ot[:])
                return out

            # ---------------- initial convs ----------------
            x1in = rg.tile([128, 8, NL], F16, tag="R1")
            nc.sync.dma_start(x1in[:], din["x1s"].rearrange("(t p) n -> p t n", p=128))
            x2in = rg.tile([128, 8, NL], F16, tag="R3")
            nc.sync.dma_start(x2in[:], din["x2s"].rearrange("(t p) n -> p t n", p=128))
            x1 = conv("w_conv1", "b_conv1", x1in, 8, 8, rg.tile([128, 8, NL], F16, tag="R2", name="x1a"))
            x2 = conv("w_conv1", "b_conv1", x2in, 8, 8, rg.tile([128, 8, NL], F16, tag="R4", name="x2a"))

            x2d = {0: dr.tile([128, 4, NL], F16, tag="x2da", name="x2da"),
                   1: dr.tile([128, 4, NL], F16, tag="x2db", name="x2db")}

            for s in (1, 2, 3, 4):
                C = CS[s]
                cc = C // 128
                nh = max(1, C // 512)          # 512-wide halves of C
                hw = 512 if C >= 512 else C
                if s >= 2:
                    x2 = rg.tile([128, cc, NL], F16, tag="R5")
                    nc.sync.dma_start(x2[:], x2d[s % 2][:, :cc, :])

                # ---- phase A: q, vT, k convs; x2next; AllGather ----
                q = conv(f"w_q{s}", f"b_q{s}", x1, cc, cc, rg.tile([128, cc, NL], F16, tag="R3", name=f"q{s}"))

                agi = dr.tile([2, C * NL], F16, tag="agi")
                ago = dr.tile([4, 2, C * NL], F16, tag="ago")
                agik = agi[0].rearrange("(t p n) -> p t n", p=128, t=cc)
                agiv = agi[1].rearrange("(mt p c) -> p mt c", p=128, mt=QT)

                for h in range(nh):
                    wv = s1p.tile([128, cc, hw], F16, tag="WV")
                    nc.sync.dma_start(
                        wv[:], din[f"w_v{s}"][:, h * 512:h * 512 + hw]
                        .rearrange("(t p) o -> p t o", p=128))
                    for mt in range(QT):
                        ps = pc.tile([128, 512], F32, tag="CV")
                        for ci in range(cc):
                            nc.tensor.matmul(
                                ps[:, :hw], x1[:, ci, mt * 128:(mt + 1) * 128],
                                wv[:, ci, :], start=(ci == 0), stop=(ci == cc - 1))
                        vev = s2p.tile([128, 512], F16, tag="VEV")
                        nc.scalar.activation(vev[:, :hw], ps[:, :hw], AF.Copy)
                        nc.sync.dma_start(agiv[:, mt, h * 512:h * 512 + hw], vev[:, :hw])

                conv(f"w_k{s}", f"b_k{s}", x2, cc, cc, None, out_dram=agik)
                if s < 4:
                    ncc = CS[s + 1] // 128
                    x2nd = x2d[(s + 1) % 2]
                    conv(f"w_conv{s + 1}", f"b_conv{s + 1}", x2, cc, ncc, None,
                         out_dram=x2nd)

                nc.gpsimd.collective_compute(
                    "AllGather", ALU.bypass, replica_groups=GROUPS,
                    ins=[agi.opt()], outs=[ago.opt()])

                x1d = dr.tile([128, 8, NL], F16, tag="x1d")
                nc.sync.dma_start(x1d[:, :cc, :], x1[:])

                # ---- phase B: attention ----
                gamma = load_b(f"gamma{s}", 1)
                gbv = load_b(f"gbv{s}", cc)
                Osb = rg.tile([128, QT, C], F16, tag="R5")
                Mx = st.tile([128, QT], F32, tag="Mx")
                Sm = st.tile([128, QT], F32, tag="Sm")
                y = rg.tile([128, cc, NL], F16, tag="R4")

                for mq in range(4):
                    Ksb = rg.tile([128, cc, NL], F16, tag="R1")
                    nc.sync.dma_start(Ksb[:], ago[mq, 0].rearrange("(t p n) -> p t n", p=128, t=cc))
                    VTsb = rg.tile([128, QT, C], F16, tag="R2")
                    nc.sync.dma_start(VTsb[:], ago[mq, 1].rearrange("(mt p c) -> p mt c", p=128, mt=QT))
                    for qt in range(QT):
                        sp = []
                        for ic, (off, wd) in enumerate(NCH):
                            p = pp.tile([128, 512], F32, tag=f"S{ic}")
                            for ci in range(cc):
                                nc.tensor.matmul(
                                    p[:, :wd], q[:, ci, qt * 128:(qt + 1) * 128],
                                    Ksb[:, ci, off:off + wd],
                                    start=(ci == 0), stop=(ci == cc - 1))
                            sp.append(p)
                        if mq == 3:
                            nc.vector.memset(sp[2][:, PADL - 1024:512], -1e30)
                            nc.vector.memset(sp[3][:, 0:256], -1e30)
                        t4 = st.tile([128, 4], F32, tag="t4")
                        for ic, (off, wd) in enumerate(NCH):
                            nc.vector.tensor_reduce(t4[:, ic:ic + 1], sp[ic][:, :wd],
                                                    axis=AX.X, op=ALU.max)
                        mx = st.tile([128, 1], F32, tag="mx")
                        nc.vector.tensor_reduce(mx[:], t4[:], axis=AX.X, op=ALU.max)
                        negM = st.tile([128, 1], F32, tag="negM")
                        alpha = st.tile([128, 1], F32, tag="alpha")
                        if mq == 0:
                            nc.vector.tensor_copy(Mx[:, qt:qt + 1], mx[:])
                            nc.vector.tensor_scalar_mul(negM[:], mx[:], -1.0)
                        else:
                            newM = st.tile([128, 1], F32, tag="newM")
                            nc.vector.tensor_tensor(newM[:], Mx[:, qt:qt + 1], mx[:], op=ALU.max)
                            dM = st.tile([128, 1], F32, tag="dM")
                            nc.vector.tensor_sub(dM[:], Mx[:, qt:qt + 1], newM[:])
                            nc.scalar.activation(alpha[:], dM[:], AF.Exp)
                            nc.vector.tensor_copy(Mx[:, qt:qt + 1], newM[:])
                            nc.vector.tensor_scalar_mul(negM[:], newM[:], -1.0)
                        A = s1p.tile([128, NL], F16, tag="A")
                        r4 = st.tile([128, 4], F32, tag="r4")
                        for ic, (off, wd) in enumerate(NCH):
                            nc.scalar.activation(A[:, off:off + wd], sp[ic][:, :wd], AF.Exp,
                                                 bias=negM[:], accum_out=r4[:, ic:ic + 1])
                        rs = st.tile([128, 1], F32, tag="rs")
                        nc.vector.tensor_reduce(rs[:], r4[:], axis=AX.X, op=ALU.add)
                        if mq == 0:
                            nc.vector.tensor_copy(Sm[:, qt:qt + 1], rs[:])
                        else:
                            nc.vector.scalar_tensor_tensor(
                                Sm[:, qt:qt + 1], Sm[:, qt:qt + 1], alpha[:], rs[:],
                                ALU.mult, ALU.add)
                        AT = s1p.tile([128, QT, 128], F16, tag="AT")
                        for pk in range(4):
                            nmt = 4 if pk < 3 else 2
                            pt = pp.tile([128, 512], F32, tag="S0")
                            for j in range(nmt):
                                mt = pk * 4 + j
                                nc.tensor.matmul(pt[:, j * 128:(j + 1) * 128],
                                                 A[:, mt * 128:(mt + 1) * 128], ident[:],
                                                 start=True, stop=True)
                            nc.vector.tensor_copy(
                                AT[:, pk * 4:pk * 4 + nmt, :].rearrange("p a b -> p (a b)"),
                                pt[:, :nmt * 128])
                        for h in range(nh):
                            po = pp.tile([128, 512], F32, tag=f"O{h}")
                            for mt in range(QT):
                                nc.tensor.matmul(po[:, :hw], AT[:, mt, :],
                                                 VTsb[:, mt, h * 512:h * 512 + hw],
                                                 start=(mt == 0), stop=(mt == QT - 1))
                            osl = Osb[:, qt, h * 512:h * 512 + hw]
                            if mq == 0:
                                nc.vector.tensor_copy(osl, po[:, :hw])
                            else:
                                nc.vector.scalar_tensor_tensor(
                                    osl, osl, alpha[:], po[:, :hw], ALU.mult, ALU.add)
                # finalize: y = x1 + (gamma/sum) * O^T + gamma*b_v
                for qt in range(QT):
                    rec = st.tile([128, 1], F32, tag="rec")
                    nc.vector.reciprocal(rec[:], Sm[:, qt:qt + 1])
                    fac = st.tile([128, 1], F32, tag="fac")
                    nc.vector.tensor_tensor(fac[:], rec[:], gamma[:], op=ALU.mult)
                    Of = s1p.tile([128, C], F16, tag="Of")
                    nc.scalar.activation(Of[:], Osb[:, qt, :], AF.Copy, scale=fac[:])
                    x1q = s1p.tile([128, cc, 128], F16, tag="x1q")
                    nc.sync.dma_start(x1q[:], x1d[:, :cc, qt * 128:(qt + 1) * 128])
                    for ct in range(cc):
                        py = pp.tile([128, 512], F32, tag="S0")
                        nc.tensor.matmul(py[:, :128], Of[:, ct * 128:(ct + 1) * 128], ident[:],
                                         start=True, stop=True)
                        nc.vector.scalar_tensor_tensor(
                            y[:, ct, qt * 128:(qt + 1) * 128], py[:, :128],
                            gbv[:, ct:ct + 1], x1q[:, ct, :], ALU.add, ALU.add)

                # ---- phase C: resblock ----
                norm = s >= 2
                if norm:
                    for ct in range(cc):
                        nc.vector.tensor_tensor(y[:, ct, :], y[:, ct, :], maskh[:], op=ALU.mult)

                _stc = [0]

                def inorm_stats(x_t):
                    _stc[0] += 1
                    stt = st.tile([128, cc, 2], F32, tag="stt")
                    sq = s1p.tile([128, NL], F32, tag="SCR7")
                    for ct in range(cc):
                        nc.vector.tensor_reduce(stt[:, ct, 0:1], x_t[:, ct, :],
                                                axis=AX.X, op=ALU.add)
                        nc.scalar.activation(sq[:], x_t[:, ct, :], AF.Square,
                                             accum_out=stt[:, ct, 1:2])
                    sdi = dr.tile([128, 8, 2], F32, tag="sdi")
                    sdo = dr.tile([128, 8, 2], F32, tag="sdo")
                    nc.sync.dma_start(sdi[:, :cc, :], stt[:])
                    nc.gpsimd.collective_compute(
                        "AllReduce", ALU.add, replica_groups=GROUPS,
                        ins=[sdi.opt()], outs=[sdo.opt()])
                    st2 = st.tile([128, cc, 2], F32, tag="st2")
                    nc.sync.dma_start(st2[:], sdo[:, :cc, :])
                    mean = st.tile([128, cc], F32, tag=f"mean{_stc[0] % 2}")
                    nc.vector.tensor_scalar_mul(mean[:], st2[:, :, 0], 1.0 / N)
                    ex2 = st.tile([128, cc], F32, tag="ex2")
                    nc.vector.tensor_scalar_mul(ex2[:], st2[:, :, 1], 1.0 / N)
                    var = st.tile([128, cc], F32, tag="var")
                    nc.vector.tensor_tensor(var[:], mean[:], mean[:], op=ALU.mult)
                    nc.vector.tensor_sub(var[:], ex2[:], var[:])
                    nc.vector.tensor_scalar_add(var[:], var[:], 1e-5)
                    std = st.tile([128, cc], F32, tag="std")
                    nc.scalar.activation(std[:], var[:], AF.Sqrt)
                    inv = st.tile([128, cc], F32, tag=f"inv{_stc[0] % 2}")
                    nc.vector.reciprocal(inv[:], std[:])
                    return mean, inv

                def spadain_relu(x_t, wkey, bkey, mean, inv, out, mask_out):
                    wsp = cn.tile([3, 1024], F16, tag="wsp")
                    nc.sync.dma_start(wsp[:, :C], din[wkey][:])
                    bsp = cn.tile([3, 1024], F16, tag="bsp")
                    nc.sync.dma_start(bsp[:, :C], din[bkey][:])
                    bw = load_b("b_" + wkey[2:], cc)
                    bb = load_b("b_" + bkey[2:], cc)
                    for ct in range(cc):
                        if mean is not None:
                            xh = s1p.tile([128, NL], F32, tag="SCR7")
                            nc.vector.tensor_scalar(
                                xh[:], x_t[:, ct, :], mean[:, ct:ct + 1], inv[:, ct:ct + 1],
                                op0=ALU.subtract, op1=ALU.mult)
                            src = xh
                        for off, wd in NCH:
                            pw = pc.tile([128, 512], F32, tag="CV")
                            nc.tensor.matmul(pw[:, :wd], wsp[:, ct * 128:(ct + 1) * 128],
                                             adds[:, off:off + wd], start=True, stop=True)
                            pb = pc.tile([128, 512], F32, tag="CV")
                            nc.tensor.matmul(pb[:, :wd], bsp[:, ct * 128:(ct + 1) * 128],
                                             adds[:, off:off + wd], start=True, stop=True)
                            t2 = s1p.tile([128, 512], F32, tag="t2")
                            nc.vector.scalar_tensor_tensor(
                                t2[:, :wd], pw[:, :wd], bw[:, ct:ct + 1],
                                (src[:, off:off + wd] if mean is not None
                                 else x_t[:, ct, off:off + wd]),
                                ALU.add, ALU.mult)
                            t3 = s1p.tile([128, 512], F32, tag="t3")
                            nc.vector.scalar_tensor_tensor(
                                t3[:, :wd], pb[:, :wd], bb[:, ct:ct + 1], t2[:, :wd],
                                ALU.add, ALU.add)
                            nc.scalar.activation(out[:, ct, off:off + wd], t3[:, :wd], AF.Relu)
                        if mask_out:
                            nc.vector.tensor_tensor(out[:, ct, :], out[:, ct, :], maskh[:],
                                                    op=ALU.mult)
                    return out

                if norm:
                    mean_x, inv_x = inorm_stats(y)
                else:
                    mean_x, inv_x = None, None
                u1 = rg.tile([128, cc, NL], F16, tag="R1")
                spadain_relu(y, f"w_sp1w{s}", f"w_sp1b{s}", mean_x, inv_x, u1, norm)
                out1 = conv(f"w_rc1{s}", None if norm else f"b_rc1{s}", u1, cc, cc,
                            rg.tile([128, cc, NL], F16, tag="R2", name=f"o1_{s}"))
                if norm:
                    mean_o, inv_o = inorm_stats(out1)
                else:
                    mean_o, inv_o = None, None
                u2 = rg.tile([128, cc, NL], F16, tag="R1")
                spadain_relu(out1, f"w_sp2w{s}", f"w_sp2b{s}", mean_o, inv_o, u2, False)
                o2 = conv(f"w_rc2{s}", f"b_rc2{s}", u2, cc, cc,
                          rg.tile([128, cc, NL], F16, tag="R3", name=f"o2_{s}"))
                ur = rg.tile([128, cc, NL], F16, tag="R1")
                spadain_relu(y, f"w_sprw{s}", f"w_sprb{s}", mean_x, inv_x, ur, False)
                orr = conv(f"w_rcr{s}", f"b_rcr{s}", ur, cc, cc,
                           rg.tile([128, cc, NL], F16, tag="R2", name=f"or_{s}"))
                xn = rg.tile([128, cc, NL], F16, tag="R1")
                for ct in range(cc):
                    nc.vector.tensor_add(xn[:, ct, :], o2[:, ct, :], orr[:, ct, :])

                if s < 4:
                    x1 = conv(f"w_conv{s + 1}", f"b_conv{s + 1}", xn, cc, CS[s + 1] // 128,
                              rg.tile([128, CS[s + 1] // 128, NL], F16, tag="R2", name=f"x1n{s}"))
                else:
                    w5 = wp.tile([128, 2, 128], F16, tag="W")
                    nc.sync.dma_start(w5[:, :, :3],
                                      din["w_conv5"].rearrange("(t p) o -> p t o", p=128))
                    b5 = load_b("b_conv5", 1)
                    ot = s1p.tile([128, NL], F32, tag="SCR7")
                    for off, wd in NCH:
                        ps = pc.tile([128, 512], F32, tag="CV")
                        for ci in range(2):
                            nc.tensor.matmul(ps[:3, :wd], w5[:, ci, :3],
                                             xn[:, ci, off:off + wd],
                                             start=(ci == 0), stop=(ci == 1))
                        nc.scalar.activation(ot[:3, off:off + wd], ps[:3, :wd], AF.Tanh,
                                             bias=b5[:3, 0:1])
                    o2t = s1p.tile([128, NL], F32, tag="A")
                    nc.vector.tensor_scalar_mul(o2t[:3, :], ot[:3, :], 2.0)
                    nc.sync.dma_start(out_d[:], o2t[:3, :])

    nc.finalize()
    _cache["nc"] = nc
    return nc


def _prep_inputs(x1_f, x2_f, addition, params):
    f16, f32 = np.float16, np.float32
    w = {}

    def wt(name, p):
        w[name] = np.ascontiguousarray(np.asarray(p["w"], f32).T).astype(f16)

    def bt(name, p, key="b"):
        arr = np.asarray(p[key], f32) if key else np.asarray(p, f32)
        C = arr.shape[0]
        if C >= 128:
            w[name] = np.ascontiguousarray(arr.reshape(C // 128, 128).T)
        else:
            b = np.zeros((128, 1), f32)
            b[:C, 0] = arr
            w[name] = b

    for i, key in enumerate(["conv1", "conv2", "conv3", "conv4", "conv5"], 1):
        wt(f"w_conv{i}", params[key])
        bt(f"b_conv{i}", params[key])
    for s, key in enumerate(["cgp1", "cgp2", "cgp3", "cgp4"], 1):
        cg = params[key]
        wt(f"w_q{s}", cg["q"]); bt(f"b_q{s}", cg["q"])
        wt(f"w_k{s}", cg["k"]); bt(f"b_k{s}", cg["k"])
        wt(f"w_v{s}", cg["v"])
        gamma = float(np.asarray(cg["gamma"]))
        w[f"gamma{s}"] = np.full((128, 1), gamma, f32)
        bt(f"gbv{s}", gamma * np.asarray(cg["v"]["b"], f32), key=None)
    for s, key in enumerate(["rb1", "rb2", "rb3", "rb4"], 1):
        rb = params[key]
        for t in ("sp1", "sp2", "spr"):
            wt(f"w_{t}w{s}", rb[t]["w"]); bt(f"b_{t}w{s}", rb[t]["w"])
            wt(f"w_{t}b{s}", rb[t]["b"]); bt(f"b_{t}b{s}", rb[t]["b"])
        for t_out, t_in in [("rc1", "conv1"), ("rc2", "conv2"), ("rcr", "convr")]:
            wt(f"w_{t_out}{s}", rb[t_in]); bt(f"b_{t_out}{s}", rb[t_in])
    w["ident"] = np.eye(128, dtype=f16)

    x1_f = np.asarray(x1_f, f32)
    x2_f = np.asarray(x2_f, f32)
    addition = np.asarray(addition, f32)
    in_maps = []
    for core in range(8):
        b, r = core // 4, core % 4
        lo = r * NL
        hi = min(lo + NL, N)
        wd = hi - lo
        m = dict(w)
        for nm, src in [("x1s", x1_f[b]), ("x2s", x2_f[b])]:
            t = np.zeros((1024, NL), f16)
            t[:, :wd] = src[:, lo:hi].astype(f16)
            m[nm] = t
        t = np.zeros((3, NL), f16)
        t[:, :wd] = addition[b][:, lo:hi].astype(f16)
        m["adds"] = t
        mk = np.zeros((128, NL), f16)
        mk[:, :wd] = 1.0
        m["maskh"] = mk
        in_maps.append(m)
    return in_maps


def kernel(x1_f, x2_f, addition, params):
    from concourse.bass_utils import run_bass_kernel_spmd
    nc = _build()
    in_maps = _prep_inputs(x1_f, x2_f, addition, params)
    res = run_bass_kernel_spmd(nc, in_maps, core_ids=list(range(8)))
    out = np.zeros((B, 3, N), np.float32)
    for core in range(8):
        b, r = core // 4, core % 4
        lo = r * NL
        hi = min(lo + NL, N)
        out[b][:, lo:hi] = res.results[core]["out"][:, :hi - lo]
    return out


# revision 10
# speedup vs baseline: 2.1378x; 2.1378x over previous
"""Trainium2 Bass kernel for nn_Decoder_90443421319294.

Sharding: 2 batches x 4-way sequence-parallel over query rows (8 NeuronCores).
Within each 4-core group, K and V^T are computed on row shards and AllGathered;
attention (S = Q^T K, unscaled softmax over keys, O = A @ V^T) runs flash-style
over 4 key quarters with online softmax. InstanceNorm stats use AllReduce.
All matmuls use fp16 inputs with fp32 PSUM accumulation (validated: ~7.5e-3
relative absmax vs fp64 reference end-to-end).

Self-contained: hardcodes all shapes; call kernel(**setup_inputs()).
"""
import numpy as np

N = 6890          # real positions
NP = 7168         # padded positions (56 * 128)
NL = 1792         # positions per core (NP / 4)
QT = 14           # query tiles of 128 per core
B = 2
CS = {1: 1024, 2: 512, 3: 512, 4: 256}
NCH = [(0, 512), (512, 512), (1024, 512), (1536, 256)]  # 512-chunks of 1792
PADL = N - 3 * NL   # 1514: first pad column (local) in the 4th quarter
GROUPS = [[0, 1, 2, 3], [4, 5, 6, 7]]

_cache = {}


def _build():
    if "nc" in _cache:
        return _cache["nc"]
    import concourse.bacc as bacc
    import concourse.mybir as mybir
    import concourse.tile as tile

    F32 = mybir.dt.float32
    F16 = mybir.dt.float16
    AF = mybir.ActivationFunctionType
    ALU = mybir.AluOpType
    AX = mybir.AxisListType

    nc = bacc.Bacc("TRN2", target_bir_lowering=False, debug=False, num_devices=8)

    din = {}

    def inp(name, shape, dt):
        din[name] = nc.dram_tensor(name, shape, dt, kind="ExternalInput")

    inp("x1s", [1024, NL], F16)
    inp("x2s", [1024, NL], F16)
    inp("adds", [3, NL], F16)
    inp("maskh", [128, NL], F16)
    inp("ident", [128, 128], F16)
    for i, (ci, co) in enumerate([(1024, 1024), (1024, 512), (512, 512), (512, 256), (256, 3)], 1):
        inp(f"w_conv{i}", [ci, co], F16)
        inp(f"b_conv{i}", [128, max(1, co // 128)], F32)
    for s in (1, 2, 3, 4):
        C = CS[s]
        cc = C // 128
        for t in ("q", "k", "v"):
            inp(f"w_{t}{s}", [C, C], F16)
        inp(f"b_q{s}", [128, cc], F32)
        inp(f"b_k{s}", [128, cc], F32)
        inp(f"gbv{s}", [128, cc], F32)
        inp(f"gamma{s}", [128, 1], F32)
        for t in ("sp1w", "sp1b", "sp2w", "sp2b", "sprw", "sprb"):
            inp(f"w_{t}{s}", [3, C], F16)
            inp(f"b_{t}{s}", [128, cc], F32)
        for t in ("rc1", "rc2", "rcr"):
            inp(f"w_{t}{s}", [C, C], F16)
            inp(f"b_{t}{s}", [128, cc], F32)
    out_d = nc.dram_tensor("out", [3, NL], F32, kind="ExternalOutput")

    with tile.TileContext(nc) as tc:
        with (
            tc.tile_pool(name="cn", bufs=1) as cn,
            tc.tile_pool(name="wp", bufs=2) as wp,
            tc.tile_pool(name="rg", bufs=1) as rg,
            tc.tile_pool(name="s1", bufs=1) as s1p,
            tc.tile_pool(name="s2", bufs=2) as s2p,
            tc.tile_pool(name="st", bufs=1) as st,
            tc.tile_pool(name="pp", bufs=1, space="PSUM") as pp,
            tc.tile_pool(name="pc", bufs=2, space="PSUM") as pc,
            tc.tile_pool(name="dr", bufs=1, space="DRAM") as dr,
        ):
            ident = cn.tile([128, 128], F16, tag="ident")
            nc.sync.dma_start(ident[:], din["ident"][:])
            maskh = cn.tile([128, NL], F16, tag="maskh")
            nc.sync.dma_start(maskh[:], din["maskh"][:])
            adds = cn.tile([3, NL], F16, tag="adds")
            nc.sync.dma_start(adds[:], din["adds"][:])

            def load_b(name, cc):
                b = st.tile([128, cc], F32, tag=f"B_{name}")
                nc.sync.dma_start(b[:], din[name][:])
                return b

            def conv(w_name, b_name, x, ci_t, co_t, out, relu=False, out_dram=None):
                """x [128, ci_t, NL] f16 -> out [128, co_t, NL] f16 (SBUF tile),
                or stream per-co to out_dram [128, co_t, NL] via small buffers."""
                b = load_b(b_name, co_t) if b_name else None
                for co in range(co_t):
                    w = wp.tile([128, ci_t, 128], F16, tag="W")
                    nc.sync.dma_start(
                        w[:], din[w_name][:, co * 128:(co + 1) * 128]
                        .rearrange("(t p) o -> p t o", p=128))
                    if out_dram is not None:
                        ot = s2p.tile([128, NL], F16, tag="KEV")
                    for off, wd in NCH:
                        ps = pc.tile([128, 512], F32, tag="CV")
                        for ci in range(ci_t):
                            nc.tensor.matmul(
                                ps[:, :wd], w[:, ci, :], x[:, ci, off:off + wd],
                                start=(ci == 0), stop=(ci == ci_t - 1))
                        dst = ot[:, off:off + wd] if out_dram is not None else out[:, co, off:off + wd]
                        if b is None:
                            nc.scalar.activation(dst, ps[:, :wd], AF.Relu if relu else AF.Copy)
                        else:
                            nc.scalar.activation(dst, ps[:, :wd],
                                                 AF.Relu if relu else AF.Identity,
                                                 bias=b[:, co:co + 1])
                    if out_dram is not None:
                        nc.sync.dma_start(out_dram[:, co, :], ot[:])
                return out

            # ---------------- initial convs ----------------
            x1in = rg.tile([128, 8, NL], F16, tag="R1")
            nc.sync.dma_start(x1in[:], din["x1s"].rearrange("(t p) n -> p t n", p=128))
            x2in = rg.tile([128, 8, NL], F16, tag="R3")
            nc.sync.dma_start(x2in[:], din["x2s"].rearrange("(t p) n -> p t n", p=128))
            x1 = conv("w_conv1", "b_conv1", x1in, 8, 8, rg.tile([128, 8, NL], F16, tag="R2", name="x1a"))
            x2 = conv("w_conv1", "b_conv1", x2in, 8, 8, rg.tile([128, 8, NL], F16, tag="R4", name="x2a"))

            x2d = {0: dr.tile([128, 4, NL], F16, tag="x2da", name="x2da"),
                   1: dr.tile([128, 4, NL], F16, tag="x2db", name="x2db")}

            for s in (1, 2, 3, 4):
                C = CS[s]
                cc = C // 128
                nh = max(1, C // 512)          # 512-wide halves of C
                hw = 512 if C >= 512 else C
                if s >= 2:
                    x2 = rg.tile([128, cc, NL], F16, tag="R5")
                    nc.sync.dma_start(x2[:], x2d[s % 2][:, :cc, :])

                # ---- phase A: q, vT, k convs; x2next; AllGather ----
                q = conv(f"w_q{s}", f"b_q{s}", x1, cc, cc, rg.tile([128, cc, NL], F16, tag="R3", name=f"q{s}"))

                agi = dr.tile([2, C * NL], F16, tag="agi")
                ago = dr.tile([4, 2, C * NL], F16, tag="ago")
                agik = agi[0].rearrange("(t p n) -> p t n", p=128, t=cc)
                agiv = agi[1].rearrange("(mt p c) -> p mt c", p=128, mt=QT)

                for h in range(nh):
                    wv = s1p.tile([128, cc, hw], F16, tag="WV")
                    nc.sync.dma_start(
                        wv[:], din[f"w_v{s}"][:, h * 512:h * 512 + hw]
                        .rearrange("(t p) o -> p t o", p=128))
                    for mt in range(QT):
                        ps = pc.tile([128, 512], F32, tag="CV")
                        for ci in range(cc):
                            nc.tensor.matmul(
                                ps[:, :hw], x1[:, ci, mt * 128:(mt + 1) * 128],
                                wv[:, ci, :], start=(ci == 0), stop=(ci == cc - 1))
                        vev = s2p.tile([128, 512], F16, tag="VEV")
                        nc.scalar.activation(vev[:, :hw], ps[:, :hw], AF.Copy)
                        nc.sync.dma_start(agiv[:, mt, h * 512:h * 512 + hw], vev[:, :hw])

                conv(f"w_k{s}", f"b_k{s}", x2, cc, cc, None, out_dram=agik)
                if s < 4:
                    ncc = CS[s + 1] // 128
                    x2nd = x2d[(s + 1) % 2]
                    conv(f"w_conv{s + 1}", f"b_conv{s + 1}", x2, cc, ncc, None,
                         out_dram=x2nd)

                nc.gpsimd.collective_compute(
                    "AllGather", ALU.bypass, replica_groups=GROUPS,
                    ins=[agi.opt()], outs=[ago.opt()])

                x1d = dr.tile([128, 8, NL], F16, tag="x1d")
                nc.sync.dma_start(x1d[:, :cc, :], x1[:])

                # ---- phase B: attention ----
                gamma = load_b(f"gamma{s}", 1)
                gbv = load_b(f"gbv{s}", cc)
                Osb = rg.tile([128, QT, C], F16, tag="R5")
                Mx = st.tile([128, QT], F32, tag="Mx")
                Sm = st.tile([128, QT], F32, tag="Sm")
                y = rg.tile([128, cc, NL], F16, tag="R4")

                for mq in range(4):
                    Ksb = rg.tile([128, cc, NL], F16, tag="R1")
                    nc.sync.dma_start(Ksb[:], ago[mq, 0].rearrange("(t p n) -> p t n", p=128, t=cc))
                    VTsb = rg.tile([128, QT, C], F16, tag="R2")
                    nc.sync.dma_start(VTsb[:], ago[mq, 1].rearrange("(mt p c) -> p mt c", p=128, mt=QT))
                    for qt in range(QT):
                        sp = []
                        for ic, (off, wd) in enumerate(NCH):
                            p = pp.tile([128, 512], F32, tag=f"S{ic}")
                            for ci in range(cc):
                                nc.tensor.matmul(
                                    p[:, :wd], q[:, ci, qt * 128:(qt + 1) * 128],
                                    Ksb[:, ci, off:off + wd],
                                    start=(ci == 0), stop=(ci == cc - 1))
                            sp.append(p)
                        if mq == 3:
                            nc.vector.memset(sp[2][:, PADL - 1024:512], -1e30)
                            nc.vector.memset(sp[3][:, 0:256], -1e30)
                        t4 = st.tile([128, 4], F32, tag="t4")
                        for ic, (off, wd) in enumerate(NCH):
                            nc.vector.tensor_reduce(t4[:, ic:ic + 1], sp[ic][:, :wd],
                                                    axis=AX.X, op=ALU.max)
                        mx = st.tile([128, 1], F32, tag="mx")
                        nc.vector.tensor_reduce(mx[:], t4[:], axis=AX.X, op=ALU.max)
                        negM = st.tile([128, 1], F32, tag="negM")
                        alpha = st.tile([128, 1], F32, tag="alpha")
                        if mq == 0:
                            nc.vector.tensor_copy(Mx[:, qt:qt + 1], mx[:])
                            nc.vector.tensor_scalar_mul(negM[:], mx[:], -1.0)
                        else:
                            newM = st.tile([128, 1], F32, tag="newM")
                            nc.vector.tensor_tensor(newM[:], Mx[:, qt:qt + 1], mx[:], op=ALU.max)
                            dM = st.tile([128, 1], F32, tag="dM")
                            nc.vector.tensor_sub(dM[:], Mx[:, qt:qt + 1], newM[:])
                            nc.scalar.activation(alpha[:], dM[:], AF.Exp)
                            nc.vector.tensor_copy(Mx[:, qt:qt + 1], newM[:])
                            nc.vector.tensor_scalar_mul(negM[:], newM[:], -1.0)
                        A = s1p.tile([128, NL], F16, tag="A")
                        r4 = st.tile([128, 4], F32, tag="r4")
                        for ic, (off, wd) in enumerate(NCH):
                            nc.scalar.activation(A[:, off:off + wd], sp[ic][:, :wd], AF.Exp,
                                                 bias=negM[:], accum_out=r4[:, ic:ic + 1])
                        rs = st.tile([128, 1], F32, tag="rs")
                        nc.vector.tensor_reduce(rs[:], r4[:], axis=AX.X, op=ALU.add)
                        if mq == 0:
                            nc.vector.tensor_copy(Sm[:, qt:qt + 1], rs[:])
                        else:
                            nc.vector.scalar_tensor_tensor(
                                Sm[:, qt:qt + 1], Sm[:, qt:qt + 1], alpha[:], rs[:],
                                ALU.mult, ALU.add)
                        AT = s1p.tile([128, QT, 128], F16, tag="AT")
                        for pk in range(4):
                            nmt = 4 if pk < 3 else 2
                            pt = pp.tile([128, 512], F32, tag="S0")
                            for j in range(nmt):
                                mt = pk * 4 + j
                                nc.tensor.matmul(pt[:, j * 128:(j + 1) * 128],
                                                 A[:, mt * 128:(mt + 1) * 128], ident[:],
                                                 start=True, stop=True)
                            nc.vector.tensor_copy(
                                AT[:, pk * 4:pk * 4 + nmt, :].rearrange("p a b -> p (a b)"),
                                pt[:, :nmt * 128])
                        for h in range(nh):
                            po = pp.tile([128, 512], F32, tag=f"O{h}")
                            for mt in range(QT):
                                nc.tensor.matmul(po[:, :hw], AT[:, mt, :],
                                                 VTsb[:, mt, h * 512:h * 512 + hw],
                                                 start=(mt == 0), stop=(mt == QT - 1))
                            osl = Osb[:, qt, h * 512:h * 512 + hw]
                            if mq == 0:
                                nc.vector.tensor_copy(osl, po[:, :hw])
                            else:
                                nc.vector.scalar_tensor_tensor(
                                    osl, osl, alpha[:], po[:, :hw], ALU.mult, ALU.add)
                # finalize: y = x1 + (gamma/sum) * O^T + gamma*b_v
                for qt in range(QT):
                    rec = st.tile([128, 1], F32, tag="rec")
                    nc.vector.reciprocal(rec[:], Sm[:, qt:qt + 1])
                    fac = st.tile([128, 1], F32, tag="fac")
                    nc.vector.tensor_tensor(fac[:], rec[:], gamma[:], op=ALU.mult)
                    Of = s1p.tile([128, C], F16, tag="Of")
                    nc.scalar.activation(Of[:], Osb[:, qt, :], AF.Copy, scale=fac[:])
                    x1q = s1p.tile([128, cc, 128], F16, tag="x1q")
                    nc.sync.dma_start(x1q[:], x1d[:, :cc, qt * 128:(qt + 1) * 128])
                    for ct in range(cc):
                        py = pp.tile([128, 512], F32, tag="S0")
                        nc.tensor.matmul(py[:, :128], Of[:, ct * 128:(ct + 1) * 128], ident[:],
                                         start=True, stop=True)
                        nc.vector.scalar_tensor_tensor(
                            y[:, ct, qt * 128:(qt + 1) * 128], py[:, :128],
                            gbv[:, ct:ct + 1], x1q[:, ct, :], ALU.add, ALU.add)

                # ---- phase C: resblock ----
                norm = s >= 2
                if norm:
                    for ct in range(cc):
                        nc.vector.tensor_tensor(y[:, ct, :], y[:, ct, :], maskh[:], op=ALU.mult)

                _stc = [0]

                def inorm_stats(x_t):
                    _stc[0] += 1
                    stt = st.tile([128, cc, 2], F32, tag="stt")
                    sq = s1p.tile([128, NL], F32, tag="SCR7")
                    for ct in range(cc):
                        nc.vector.tensor_reduce(stt[:, ct, 0:1], x_t[:, ct, :],
                                                axis=AX.X, op=ALU.add)
                        nc.scalar.activation(sq[:], x_t[:, ct, :], AF.Square,
                                             accum_out=stt[:, ct, 1:2])
                    sdi = dr.tile([128, 8, 2], F32, tag="sdi")
                    sdo = dr.tile([128, 8, 2], F32, tag="sdo")
                    nc.sync.dma_start(sdi[:, :cc, :], stt[:])
                    nc.gpsimd.collective_compute(
                        "AllReduce", ALU.add, replica_groups=GROUPS,
                        ins=[sdi.opt()], outs=[sdo.opt()])
                    st2 = st.tile([128, cc, 2], F32, tag="st2")
                    nc.sync.dma_start(st2[:], sdo[:, :cc, :])
                    mean = st.tile([128, cc], F32, tag=f"mean{_stc[0] % 2}")
                    nc.vector.tensor_scalar_mul(mean[:], st2[:, :, 0], 1.0 / N)
                    ex2 = st.tile([128, cc], F32, tag="ex2")
                    nc.vector.tensor_scalar_mul(ex2[:], st2[:, :, 1], 1.0 / N)
                    var = st.tile([128, cc], F32, tag="var")
                    nc.vector.tensor_tensor(var[:], mean[:], mean[:], op=ALU.mult)
                    nc.vector.tensor_sub(var[:], ex2[:], var[:])
                    nc.vector.tensor_scalar_add(var[:], var[:], 1e-5)
                    std = st.tile([128, cc], F32, tag="std")
                    nc.scalar.activation(std[:], var[:], AF.Sqrt)
                    inv = st.tile([128, cc], F32, tag=f"inv{_stc[0] % 2}")
                    nc.vector.reciprocal(inv[:], std[:])
                    return mean, inv

                def spadain_relu(x_t, wkey, bkey, mean, inv, out, mask_out):
                    wsp = cn.tile([3, 1024], F16, tag="wsp")
                    nc.sync.dma_start(wsp[:, :C], din[wkey][:])
                    bsp = cn.tile([3, 1024], F16, tag="bsp")
                    nc.sync.dma_start(bsp[:, :C], din[bkey][:])
                    bw = load_b("b_" + wkey[2:], cc)
                    bb = load_b("b_" + bkey[2:], cc)
                    for ct in range(cc):
                        if mean is not None:
                            xh = s1p.tile([128, NL], F32, tag="SCR7")
                            nc.vector.tensor_scalar(
                                xh[:], x_t[:, ct, :], mean[:, ct:ct + 1], inv[:, ct:ct + 1],
                                op0=ALU.subtract, op1=ALU.mult)
                            src = xh
                        for off, wd in NCH:
                            pw = pc.tile([128, 512], F32, tag="CV")
                            nc.tensor.matmul(pw[:, :wd], wsp[:, ct * 128:(ct + 1) * 128],
                                             adds[:, off:off + wd], start=True, stop=True)
                            pb = pc.tile([128, 512], F32, tag="CV")
                            nc.tensor.matmul(pb[:, :wd], bsp[:, ct * 128:(ct + 1) * 128],
                                             adds[:, off:off + wd], start=True, stop=True)
                            t2 = s1p.tile([128, 512], F32, tag="t2")
                            nc.vector.scalar_tensor_tensor(
                                t2[:, :wd], pw[:, :wd], bw[:, ct:ct + 1],
                                (src[:, off:off + wd] if mean is not None
                                 else x_t[:, ct, off:off + wd]),
                                ALU.add, ALU.mult)
                            t3 = s1p.tile([128, 512], F32, tag="t3")
                            nc.vector.scalar_tensor_tensor(
                                t3[:, :wd], pb[:, :wd], bb[:, ct:ct + 1], t2[:, :wd],
                                ALU.add, ALU.add)
                            nc.scalar.activation(out[:, ct, off:off + wd], t3[:, :wd], AF.Relu)
                        if mask_out:
                            nc.vector.tensor_tensor(out[:, ct, :], out[:, ct, :], maskh[:],
                                                    op=ALU.mult)
                    return out

                if norm:
                    mean_x, inv_x = inorm_stats(y)
                else:
                    mean_x, inv_x = None, None
                u1 = rg.tile([128, cc, NL], F16, tag="R1")
                spadain_relu(y, f"w_sp1w{s}", f"w_sp1b{s}", mean_x, inv_x, u1, norm)
                out1 = conv(f"w_rc1{s}", None if norm else f"b_rc1{s}", u1, cc, cc,
                            rg.tile([128, cc, NL], F16, tag="R2", name=f"o1_{s}"))
                if norm:
                    mean_o, inv_o = inorm_stats(out1)
                else:
                    mean_o, inv_o = None, None
                u2 = rg.tile([128, cc, NL], F16, tag="R1")
                spadain_relu(out1, f"w_sp2w{s}", f"w_sp2b{s}", mean_o, inv_o, u2, False)
                o2 = conv(f"w_rc2{s}", f"b_rc2{s}", u2, cc, cc,
                          rg.tile([128, cc, NL], F16, tag="R3", name=f"o2_{s}"))
                ur = rg.tile([128, cc, NL], F16, tag="R1")
                spadain_relu(y, f"w_sprw{s}", f"w_sprb{s}", mean_x, inv_x, ur, False)
                orr = conv(f"w_rcr{s}", f"b_rcr{s}", ur, cc, cc,
                           rg.tile([128, cc, NL], F16, tag="R2", name=f"or_{s}"))
                xn = rg.tile([128, cc, NL], F16, tag="R1")
                for ct in range(cc):
                    nc.vector.tensor_add(xn[:, ct, :], o2[:, ct, :], orr[:, ct, :])

                if s < 4:
                    x1 = conv(f"w_conv{s + 1}", f"b_conv{s + 1}", xn, cc, CS[s + 1] // 128,
                              rg.tile([128, CS[s + 1] // 128, NL], F16, tag="R2", name=f"x1n{s}"))
                else:
                    w5 = wp.tile([128, 2, 128], F16, tag="W")
                    nc.sync.dma_start(w5[:, :, :3],
                                      din["w_conv5"].rearrange("(t p) o -> p t o", p=128))
                    b5 = load_b("b_conv5", 1)
                    ot = s1p.tile([128, NL], F32, tag="SCR7")
                    for off, wd in NCH:
                        ps = pc.tile([128, 512], F32, tag="CV")
                        for ci in range(2):
                            nc.tensor.matmul(ps[:3, :wd], w5[:, ci, :3],
                                             xn[:, ci, off:off + wd],
                                             start=(ci == 0), stop=(ci == 1))
                        nc.scalar.activation(ot[:3, off:off + wd], ps[:3, :wd], AF.Tanh,
                                             bias=b5[:3, 0:1])
                    o2t = s1p.tile([128, NL], F32, tag="A")
                    nc.vector.tensor_scalar_mul(o2t[:3, :], ot[:3, :], 2.0)
                    nc.sync.dma_start(out_d[:], o2t[:3, :])

    nc.finalize()
    _cache["nc"] = nc
    return nc


def _prep_inputs(x1_f, x2_f, addition, params):
    f16, f32 = np.float16, np.float32
    w = {}

    def wt(name, p):
        w[name] = np.ascontiguousarray(np.asarray(p["w"], f32).T).astype(f16)

    def bt(name, p, key="b"):
        arr = np.asarray(p[key], f32) if key else np.asarray(p, f32)
        C = arr.shape[0]
        if C >= 128:
            w[name] = np.ascontiguousarray(arr.reshape(C // 128, 128).T)
        else:
            b = np.zeros((128, 1), f32)
            b[:C, 0] = arr
            w[name] = b

    for i, key in enumerate(["conv1", "conv2", "conv3", "conv4", "conv5"], 1):
        wt(f"w_conv{i}", params[key])
        bt(f"b_conv{i}", params[key])
    for s, key in enumerate(["cgp1", "cgp2", "cgp3", "cgp4"], 1):
        cg = params[key]
        wt(f"w_q{s}", cg["q"]); bt(f"b_q{s}", cg["q"])
        wt(f"w_k{s}", cg["k"]); bt(f"b_k{s}", cg["k"])
        wt(f"w_v{s}", cg["v"])
        gamma = float(np.asarray(cg["gamma"]))
        w[f"gamma{s}"] = np.full((128, 1), gamma, f32)
        bt(f"gbv{s}", gamma * np.asarray(cg["v"]["b"], f32), key=None)
    for s, key in enumerate(["rb1", "rb2", "rb3", "rb4"], 1):
        rb = params[key]
        for t in ("sp1", "sp2", "spr"):
            wt(f"w_{t}w{s}", rb[t]["w"]); bt(f"b_{t}w{s}", rb[t]["w"])
            wt(f"w_{t}b{s}", rb[t]["b"]); bt(f"b_{t}b{s}", rb[t]["b"])
        for t_out, t_in in [("rc1", "conv1"), ("rc2", "conv2"), ("rcr", "convr")]:
            wt(f"w_{t_out}{s}", rb[t_in]); bt(f"b_{t_out}{s}", rb[t_in])
    w["ident"] = np.eye(128, dtype=f16)

    x1_f = np.asarray(x1_f, f32)
    x2_f = np.asarray(x2_f, f32)
    addition = np.asarray(addition, f32)
    in_maps = []
    for core in range(8):
        b, r = core // 4, core % 4
        lo = r * NL
        hi = min(lo + NL, N)
        wd = hi - lo
        m = dict(w)
        for nm, src in [("x1s", x1_f[b]), ("x2s", x2_f[b])]:
            t = np.zeros((1024, NL), f16)
            t[:, :wd] = src[:, lo:hi].astype(f16)
            m[nm] = t
        t = np.zeros((3, NL), f16)
        t[:, :wd] = addition[b][:, lo:hi].astype(f16)
        m["adds"] = t
        mk = np.zeros((128, NL), f16)
        mk[:, :wd] = 1.0
        m["maskh"] = mk
        in_maps.append(m)
    return in_maps


def kernel(x1_f, x2_f, addition, params):
    from concourse.bass_utils import run_bass_kernel_spmd
    nc = _build()
    in_maps = _prep_inputs(x1_f, x2_f, addition, params)
    res = None
    for attempt in range(3):
        try:
            res = run_bass_kernel_spmd(nc, in_maps, core_ids=list(range(8)))
            break
        except Exception:
            # devices occasionally report NRT_EXEC_UNIT_UNRECOVERABLE or the
            # axon worker hangs up transiently; reset jax backends and retry
            if attempt == 2:
                raise
            try:
                import jax
                jax.clear_caches()
                jax._src.api.clear_backends()
            except Exception:
                pass
            import time as _t
            _t.sleep(5)
    out = np.zeros((B, 3, N), np.float32)
    for core in range(8):
        b, r = core // 4, core % 4
        lo = r * NL
        hi = min(lo + NL, N)
        out[b][:, lo:hi] = res.results[core]["out"][:, :hi - lo]
    return out
